# revision 3
# baseline (speedup 1.0000x reference)
"""Trainium2 Bass kernel for nn_CustomLoss (argmax-distance weighted loss).

reference:
    arg = argmax(target, axis=1)              # [B]
    delta = distance[arg]                     # [B]
    err = |distance[None,:] - delta[:,None]| + 1
    loss = sum((output - target) * err) / B

Same bucket-matmul algorithm as v1 (see kernel.py); single sync HWDGE
ring (two-ring variants scramble under the Tile scheduler), but the ring
order interleaves t/o at tile granularity instead of all-t-then-all-o:

    t0(quarters) t1(halves) o0 t2 o1 t3 o2 t4 o3 t5 o4 t6 o5
    t7(quarters) o6(halves) o7(2 quarters + 4 eighths)

  - quartered t0 head: first bytes land ~8us, DVE's reduce/is_ge chain
    starts ~9 instead of 13 and finishes before the stream ends.
  - interleave means to/E tiles are consumed ~1 tile after production,
    so bufs=4 suffices (v1 needed bufs=8) -> ~173KB/partition, slack.
  - o tiles are full-tile DMAs mid-stream (bigger descriptors, fewer
    per-packet overheads); only the o7 tail is fine-grained so the
    cast+matmul chain chases it at ~0.4us grain.
  - t7 lands ~3us before o7, keeping the 2-op E-chain off the tail;
    the tail after the last byte is cast(DVE)+4 matmuls+PSUM drain.

v6 on top of v3 (DVE, 48us busy, was the critical path, ending 4us past
the DMA stream):
  - ones-column trick: is_ge writes only classes 0..3 (2048 of 2560
    outputs); class 4's lhsT column is preset to 1.0 once per physical E
    buffer, making its PSUM rows plain column sums.  The host recovers
    P[4,:] = colsum - sum(P[0..3,:]) exactly (rows are one-hot).
  - full-tile reduce/is_ge for mid tiles (t arrives as one DMA anyway);
    fewer DVE instructions, each ~0.13us of fixed overhead.
  - o6 casts on DVE too (free by then), keeping ACT's tail short.
DVE drops to ~44us busy and stays inside the stream window.

v11: with DVE off the critical path, the trace showed the matmul chain
as the tail -- tile 7's 32 blocks serialize behind tile 6's in the
single PSUM accumulator, finishing ~4us after the last cast.  Split the
accumulation: tiles 0-6 -> PSUM bank A, tile 7 -> bank B, so mm7 blocks
chase their own cast pieces concurrently with mm6's drain.  o6's casts
go back to ACT (idle by then) so mm6 isn't gated on DVE's E7 backlog.
Host sums both 5x5 blocks (accumulation is linear).
"""

from contextlib import ExitStack

import numpy as np

P = 128
C = 5
DIST = (-0.5, -0.34, 0.0, 0.34, 0.5)
B = 4194304
NCORES = 8
ROWS_PER_CORE = B // NCORES  # 524288
G = 512                      # rows per partition per tile
NTILES = ROWS_PER_CORE // (P * G)  # 8
FREE = G * C                 # 2560
HFREE = FREE // 2            # 1280
QFREE = FREE // 4            # 640
EFREE = FREE // 8            # 320
BLK = 16                     # rows-per-partition per matmul block
BLKC = BLK * C               # 80 = lhsT columns = psum partitions
NBLK = G // BLK              # 32 matmul blocks per tile
MOUT = BLKC                  # 80
NOUT = 2 * BLKC              # 160

_CACHE = {}


def _build_nc():
    import concourse.bacc as bacc
    import concourse.mybir as mybir
    import concourse.tile as tile

    F32 = mybir.dt.float32
    BF16 = mybir.dt.bfloat16

    nc = bacc.Bacc(target_bir_lowering=False)

    t_in = nc.declare_dram_parameter("t", [ROWS_PER_CORE, C], F32, isOutput=False)
    o_in = nc.declare_dram_parameter("o", [ROWS_PER_CORE, C], F32, isOutput=False)
    out = nc.declare_dram_parameter("out", [MOUT, 2 * NOUT], F32, isOutput=True)

    # row = n*(P*G) + p*G + g ; per-partition data is contiguous in DRAM
    t_tiled = t_in.rearrange("(n p g) c -> n p (g c)", p=P, g=G)
    o_tiled = o_in.rearrange("(n p g) c -> n p (g c)", p=P, g=G)

    with ExitStack() as ctx:
        tc = ctx.enter_context(tile.TileContext(nc))
        pool = ctx.enter_context(tc.tile_pool(name="work", bufs=2))
        psp = ctx.enter_context(tc.tile_pool(name="ps", bufs=1, space="PSUM"))
        outp = ctx.enter_context(tc.tile_pool(name="outp", bufs=1))
        psA = psp.tile([MOUT, NOUT], F32)
        psB = psp.tile([MOUT, NOUT], F32)

        # --- tiles -------------------------------------------------------
        tts = [None] * NTILES
        for k in (0, 1):
            tts[k] = pool.tile([P, FREE], F32, tag="thead", name=f"t{k}", bufs=2)
        for k in (2, 3, 4, 5, 6):
            tts[k] = pool.tile([P, FREE], F32, tag="tmid", name=f"t{k}", bufs=4)
        tts[7] = pool.tile([P, FREE], F32, tag="ttail", name="t7", bufs=1)

        ofs = [None] * NTILES
        for k in (0, 1, 2, 3, 4, 5):
            ofs[k] = pool.tile([P, FREE], F32, tag="omid", name=f"o{k}", bufs=3)
        for k in (6, 7):
            ofs[k] = pool.tile([P, FREE], F32, tag="otail", name=f"o{k}", bufs=2)

        tos = [
            pool.tile([P, 2 * FREE], BF16, tag="to", name=f"to{k}", bufs=5)
            for k in range(NTILES)
        ]
        Es = [
            pool.tile([P, FREE], BF16, tag="E", name=f"E{k}", bufs=5)
            for k in range(NTILES)
        ]

        # preset the c=4 ones-stripe on all physical E buffers while DVE
        # is idle in the preamble; is_ge below never writes that stripe
        for k in range(5):
            nc.gpsimd.memset(
                Es[k][:, :].rearrange("p (g c) -> p g c", c=C)[:, :, 4], 1.0
            )

        # --- DMA issue: one sync HWDGE ring, global arrival order --------
        def dma_pieces(dst, src, npieces):
            q = FREE // npieces
            for h in range(npieces):
                nc.sync.dma_start(
                    dst[:, h * q : (h + 1) * q], src[:, h * q : (h + 1) * q]
                )

        dma_pieces(tts[0], t_tiled[0], 4)
        dma_pieces(tts[1], t_tiled[1], 2)
        dma_pieces(ofs[0], o_tiled[0], 1)
        dma_pieces(tts[2], t_tiled[2], 1)
        dma_pieces(ofs[1], o_tiled[1], 1)
        dma_pieces(tts[3], t_tiled[3], 1)
        dma_pieces(ofs[2], o_tiled[2], 1)
        dma_pieces(tts[4], t_tiled[4], 1)
        dma_pieces(ofs[3], o_tiled[3], 1)
        dma_pieces(tts[5], t_tiled[5], 1)
        dma_pieces(ofs[4], o_tiled[4], 1)
        dma_pieces(tts[6], t_tiled[6], 1)
        dma_pieces(ofs[5], o_tiled[5], 1)
        dma_pieces(tts[7], t_tiled[7], 4)
        dma_pieces(ofs[6], o_tiled[6], 2)
        for h in range(2):  # o7 head quarters
            nc.sync.dma_start(
                ofs[7][:, h * QFREE : (h + 1) * QFREE],
                o_tiled[7][:, h * QFREE : (h + 1) * QFREE],
            )
        for h in range(4, 8):  # o7 tail eighths
            nc.sync.dma_start(
                ofs[7][:, h * EFREE : (h + 1) * EFREE],
                o_tiled[7][:, h * EFREE : (h + 1) * EFREE],
            )

        # --- compute: per tile, in arrival order -------------------------
        t_pieces = {0: 4, 1: 2, 2: 1, 3: 1, 4: 1, 5: 1, 6: 1, 7: 4}
        o_pieces = {0: 1, 1: 1, 2: 1, 3: 1, 4: 1, 5: 1, 6: 2}

        def cast_t(k, npieces):
            q = FREE // npieces
            for h in range(npieces):
                nc.scalar.copy(
                    tos[k][:, h * q : (h + 1) * q], tts[k][:, h * q : (h + 1) * q]
                )

        def cast_o(k, npieces):
            q = FREE // npieces
            for h in range(npieces):
                nc.scalar.copy(
                    tos[k][:, FREE + h * q : FREE + (h + 1) * q],
                    ofs[k][:, h * q : (h + 1) * q],
                )

        def e_chain(k, npieces):
            # reduce-max + is_ge per piece on the f32 t tile -> bf16 one-hot
            q = FREE // npieces
            g = G // npieces
            for h in range(npieces):
                tv = tts[k][:, h * q : (h + 1) * q].rearrange(
                    "p (g c) -> p g c", c=C
                )
                m = pool.tile([P, g], F32, tag=f"m{npieces}", name="m", bufs=2)
                nc.vector.tensor_reduce(
                    m[:, :], tv, axis=mybir.AxisListType.X, op=mybir.AluOpType.max
                )
                nc.vector.tensor_tensor(
                    Es[k][:, h * q : (h + 1) * q].rearrange(
                        "p (g c) -> p g c", c=C
                    )[:, :, 0 : C - 1],
                    tv[:, :, 0 : C - 1],
                    m[:, :].to_broadcast([P, g, C - 1]),
                    op=mybir.AluOpType.is_ge,
                )

        def matmuls(k):
            tov = tos[k][:, :].rearrange("p (s f) -> p s f", s=2)
            ps = psB if k == NTILES - 1 else psA
            for blk in range(NBLK):
                first = (k == 0 or k == NTILES - 1) and blk == 0
                last = (k == NTILES - 2 or k == NTILES - 1) and blk == NBLK - 1
                sl = slice(blk * BLKC, (blk + 1) * BLKC)
                nc.tensor.matmul(
                    ps[:, :], Es[k][:, sl], tov[:, :, sl], start=first, stop=last
                )

        for k in range(NTILES):
            cast_t(k, t_pieces[k])
            e_chain(k, max(t_pieces[k], 2))
            if k < NTILES - 1:
                cast_o(k, o_pieces[k])
            else:
                # o7 arrives in quarters+eighths at the very tail; DVE is
                # free by then and casts at 2 elem/cycle (fp32 2x_2P mode)
                for h in range(2):
                    nc.vector.tensor_copy(
                        tos[k][:, FREE + h * QFREE : FREE + (h + 1) * QFREE],
                        ofs[k][:, h * QFREE : (h + 1) * QFREE],
                    )
                for h in range(4, 8):
                    nc.vector.tensor_copy(
                        tos[k][:, FREE + h * EFREE : FREE + (h + 1) * EFREE],
                        ofs[k][:, h * EFREE : (h + 1) * EFREE],
                    )
            matmuls(k)

        # --- drain -------------------------------------------------------
        res = outp.tile([MOUT, 2 * NOUT], F32)
        nc.scalar.copy(res[:, 0:NOUT], psA[:, :])
        nc.vector.tensor_copy(res[:, NOUT : 2 * NOUT], psB[:, :])
        nc.sync.dma_start(out[:, :], res[:, :])
    nc.finalize()
    return nc


def _get_nc():
    if "nc" not in _CACHE:
        _CACHE["nc"] = _build_nc()
    return _CACHE["nc"]


def _reduce_loss(results):
    """results: iterable of per-core out arrays [80, 160] f32 -> loss."""
    dist = np.asarray(DIST, np.float64)
    W = 1.0 + np.abs(dist[None, :] - dist[:, None])  # [a, c]
    total = 0.0
    for arr in results:
        a64 = arr.astype(np.float64)  # [80, 320] = [psA(160) | psB(160)]
        Pm = sum(
            np.einsum(
                "dasdc->sac",
                a64[:, i * 160 : (i + 1) * 160].reshape(BLK, C, 2, BLK, C),
            )
            for i in (0, 1)
        )  # diag over l; [2(s=t,o), 5, 5]
        # a=4 lhsT columns were constant 1.0 -> Pm[:,4,:] holds column
        # sums; rows are one-hot so P[4] = colsum - sum(P[0..3])
        Pm[:, 4, :] = Pm[:, 4, :] - Pm[:, 0:4, :].sum(axis=1)
        total += float((W * (Pm[1] - Pm[0])).sum())
    return total / B


def kernel(output, target, distance, _want_results=False):
    from concourse.bass_utils import run_bass_kernel_spmd

    output = np.asarray(output, dtype=np.float32)
    target = np.asarray(target, dtype=np.float32)
    distance = np.asarray(distance, dtype=np.float32)
    assert output.shape == (B, C) and target.shape == (B, C)
    assert np.allclose(distance, np.asarray(DIST, np.float32)), distance

    nc = _get_nc()
    o_sh = output.reshape(NCORES, ROWS_PER_CORE, C)
    t_sh = target.reshape(NCORES, ROWS_PER_CORE, C)
    in_maps = [
        {"t": np.ascontiguousarray(t_sh[i]), "o": np.ascontiguousarray(o_sh[i])}
        for i in range(NCORES)
    ]
    res = run_bass_kernel_spmd(nc, in_maps, core_ids=list(range(NCORES)))
    loss = np.array(_reduce_loss(r["out"] for r in res.results), dtype=np.float32)
    if _want_results:
        return loss, res
    return loss
